# revision 14
# baseline (speedup 1.0000x reference)
"""AdaptiveCategoryMSA Trainium2 kernel (8 NeuronCores, data-parallel).

Host: category argmax + stable argsort + gather fused into shard step.
Device (per core = one batch-half of 8192 tokens = 64 groups of 128):
  per group: S_h = qT_h.T @ kT_h (8 heads), rowmax via segmented reduce,
  exp(scale*(s - rowmax)) with fused row-sum accum, P = E * rinv,
  PE-transpose P -> PT, Y_h = PT_h.T @ V_h, Y -> YT via 2 PE transposes,
  out = YT.T @ Wt (+bias), DMA out. All f32.
Sharding: core cx = 2*b + half handles batch b, tokens [8192*half, 8192*(half+1))
(in sorted order; groups never cross the half boundary since 8192 = 64*128).
"""
import sys
sys.path.insert(0, "/opt/trn_rl_repo")
import numpy as np

import concourse.bass as bass
import concourse.bacc as bacc
import concourse.mybir as mybir
from concourse.tile import TileContext
from concourse.bass_utils import run_bass_kernel_spmd
from concourse.bass_test_utils import axon_active

F32 = mybir.dt.float32
BF16 = mybir.dt.bfloat16

NUM_HEADS = 8
GS = 128          # group size (tokens per attention group)
NG_CORE = 64      # groups per core (8192 tokens)
C = 256           # channels
D = 32            # head dim

_cache = {}
PROFILE = False
LAST_EXEC_NS = None
LAST_TRACE = None


def _build(scale: float):
    nc = bacc.Bacc(
        "TRN2", target_bir_lowering=False, debug=False,
        enable_asserts=True, num_devices=8,
    )
    qkt = nc.dram_tensor("qkt", [NG_CORE, 4, 128, 128], F32, kind="ExternalInput")
    vg = nc.dram_tensor("vg", [NG_CORE, 128, C], BF16, kind="ExternalInput")
    wt = nc.dram_tensor("wt", [2, 128, C], BF16, kind="ExternalInput")   # proj_w.T chunks
    bfull = nc.dram_tensor("bfull", [128, C], F32, kind="ExternalInput")
    idm = nc.dram_tensor("idm", [128, 128], BF16, kind="ExternalInput")
    out = nc.dram_tensor("out", [NG_CORE, 128, C], F32, kind="ExternalOutput")

    with TileContext(nc) as tc:
        with tc.tile_pool(name="const", bufs=1) as cpool, \
             tc.tile_pool(name="sb", bufs=4) as sb, \
             tc.tile_pool(name="psS", bufs=2, space="PSUM") as psS, \
             tc.tile_pool(name="psT", bufs=2, space="PSUM") as psT, \
             tc.tile_pool(name="psY", bufs=1, space="PSUM") as psY:
            psO = psY
            wt_sb = cpool.tile([128, 2 * C], BF16)
            nc.sync.dma_start(wt_sb[:, :].rearrange("p (c n) -> p c n", c=2), wt[:, :, :].rearrange("c p n -> p c n"))
            bias_sb = cpool.tile([128, C], F32)
            nc.sync.dma_start(bias_sb[:, :], bfull[:, :])
            idm_sb = cpool.tile([128, 128], BF16)
            nc.sync.dma_start(idm_sb[:, :], idm[:, :])

            for g in range(NG_CORE):
                qk = sb.tile([128, 512], F32, tag="qk")
                nc.sync.dma_start(
                    qk[:, :].rearrange("p (c j) -> p c j", c=4),
                    qkt[g, :, :, :].rearrange("c p j -> p c j"))
                vt = sb.tile([128, C], BF16, tag="vt")
                nc.sync.dma_start(vt[:, :], vg[g, :, :])

                negmax = sb.tile([128, 8], F32, tag="negmax")
                nb = sb.tile([128, 8], F32, tag="nb")
                rs = sb.tile([128, 8], F32, tag="rs")
                rinv = sb.tile([128, 8], F32, tag="rinv")
                esb = sb.tile([128, 1024], BF16, tag="esb")
                ptsb = sb.tile([128, 1024], BF16, tag="ptsb")
                ysb = sb.tile([128, C], BF16, tag="ysb")

                # per-pair streaming: each 2-head chunk flows through
                # S -> max -> exp -> rinv -> norm -> transpose -> Y with no
                # cross-pair joins; pairs overlap through the psum pools.
                for half in range(2):
                    for pair in range(2):
                        o0 = 4 * half + 2 * pair
                        smega = psS.tile([128, 1024], F32, tag="smega")
                        for k in range(2):
                            hm = 2 * pair + k
                            lhs = qk[32 * hm:32 * hm + 32, 128 * half:128 * half + 128]
                            rhs = qk[32 * hm:32 * hm + 32,
                                     128 * (2 + half):128 * (2 + half) + 128]
                            tp = (96, 0) if hm == 3 else None
                            nc.tensor.matmul(smega[:, 512 * k:512 * k + 128],
                                             lhs, rhs, start=True, stop=True,
                                             tile_position=tp)
                        nc.vector.tensor_reduce(
                            negmax[:, o0:o0 + 2],
                            smega[:, :].rearrange("p (s j) -> p s j", s=2)[:, :, 0:128],
                            axis=mybir.AxisListType.X,
                            op=mybir.AluOpType.max, negate=True)
                        nc.vector.tensor_scalar_mul(
                            nb[:, o0:o0 + 2], negmax[:, o0:o0 + 2], scale)
                        for k in range(2):
                            h = o0 + k
                            nc.scalar.activation(
                                esb[:, 128 * h:128 * h + 128],
                                smega[:, 512 * k:512 * k + 128],
                                mybir.ActivationFunctionType.Exp,
                                bias=nb[:, h:h + 1], scale=scale,
                                accum_out=rs[:, h:h + 1])
                        nc.vector.reciprocal(rinv[:, o0:o0 + 2], rs[:, o0:o0 + 2])
                        for k in range(2):
                            h = o0 + k
                            ptp = psT.tile([128, 128], BF16, tag="ptp")
                            nc.tensor.transpose(ptp[:, :],
                                                esb[:, 128 * h:128 * h + 128],
                                                idm_sb[:, :])
                            if h % 2 == 0:
                                nc.scalar.copy(ptsb[:, 128 * h:128 * h + 128],
                                               ptp[:, :])
                            else:
                                nc.vector.tensor_copy(
                                    ptsb[:, 128 * h:128 * h + 128], ptp[:, :])
                        for k in range(2):
                            h = o0 + k
                            ypf = psY.tile([128, 256], F32, tag="yp")
                            yp = ypf[:, 0:32]
                            nc.tensor.matmul(yp[:, :],
                                             ptsb[:, 128 * h:128 * h + 128],
                                             vt[:, 32 * h:32 * h + 32],
                                             start=True, stop=True)
                            if h % 2 == 0:
                                nc.vector.tensor_scalar_mul(
                                    ysb[:, 32 * h:32 * h + 32], yp[:, :],
                                    rinv[:, h:h + 1])
                            else:
                                nc.scalar.mul(ysb[:, 32 * h:32 * h + 32],
                                              yp[:, :], rinv[:, h:h + 1])
                # Y [i, c] -> YT [c, i] chunks via PE transpose
                ytsb = sb.tile([128, 256], BF16, tag="ytsb")
                for ck in range(2):
                    ytp = psT.tile([128, 128], BF16, tag="ptp")
                    nc.tensor.transpose(ytp[:, :], ysb[:, 128 * ck:128 * ck + 128],
                                        idm_sb[:, :])
                    if ck == 0:
                        nc.scalar.copy(ytsb[:, 0:128], ytp[:, :])
                    else:
                        nc.vector.tensor_copy(ytsb[:, 128:256], ytp[:, :])
                op = psO.tile([128, 256], F32, tag="yp")
                nc.tensor.matmul(op[:, :], ytsb[:, 0:128], wt_sb[:, 0:C],
                                 start=True, stop=False)
                nc.tensor.matmul(op[:, :], ytsb[:, 128:256], wt_sb[:, C:2 * C],
                                 start=False, stop=True)
                osb = sb.tile([128, C], F32, tag="osb")
                nc.vector.tensor_add(osb[:, :], op[:, :], bias_sb[:, :])
                nc.sync.dma_start(out[g, :, :], osb[:, :])

    nc.finalize()
    return nc


def kernel(qkv, sim, proj_w, proj_b, logit_scale, h=128, w=128, **_unused):
    qkv = np.ascontiguousarray(np.asarray(qkv, dtype=np.float32))
    sim = np.asarray(sim, dtype=np.float32)
    proj_w = np.asarray(proj_w, dtype=np.float32)
    proj_b = np.asarray(proj_b, dtype=np.float32)
    ls = float(np.asarray(logit_scale, dtype=np.float32).reshape(-1)[0])
    scale = float(np.exp(min(ls, float(np.log(100.0)))))

    b, n, c3 = qkv.shape
    assert (b, n, c3) == (4, 16384, 768)

    tk = np.argmax(sim, axis=-1)                      # [b, n]
    sort_idx = np.argsort(tk, axis=-1, kind="stable")  # [b, n]

    key = round(scale, 9)
    if key not in _cache:
        _cache[key] = _build(scale)
    nc = _cache[key]

    import ml_dtypes
    wt_full = np.ascontiguousarray(proj_w.T)                       # [c, o]
    wt_in = np.ascontiguousarray(wt_full.reshape(2, 128, 256)).astype(ml_dtypes.bfloat16)
    bfull = np.ascontiguousarray(np.broadcast_to(proj_b[None, :], (128, 256)))
    idm = np.eye(128, dtype=np.float32).astype(ml_dtypes.bfloat16)

    in_maps = []
    for cx in range(8):
        bi, half = cx // 2, cx % 2
        perm = sort_idx[bi, 8192 * half:8192 * (half + 1)]
        shuf = qkv[bi][perm]                                        # [8192, 768]
        qkpart = shuf[:, 0:512].reshape(64, 128, 512)
        qkt = np.ascontiguousarray(
            qkpart.transpose(0, 2, 1).reshape(64, 4, 128, 128))
        vgv = np.ascontiguousarray(shuf[:, 512:768].reshape(64, 128, 256)).astype(ml_dtypes.bfloat16)
        in_maps.append({"qkt": qkt, "vg": vgv, "wt": wt_in,
                        "bfull": bfull, "idm": idm})

    global LAST_EXEC_NS, LAST_TRACE
    if PROFILE:
        import tempfile
        td = tempfile.mkdtemp(prefix="msa_prof_")
        res = run_bass_kernel_spmd(nc, in_maps, core_ids=list(range(8)),
                                   trace=True, tmpdir=td)
        LAST_EXEC_NS = res.exec_time_ns
        LAST_TRACE = td
    else:
        res = run_bass_kernel_spmd(nc, in_maps, core_ids=list(range(8)))

    outf = np.empty((4, 16384, 256), dtype=np.float32)
    for cx in range(8):
        bi, half = cx // 2, cx % 2
        perm = sort_idx[bi, 8192 * half:8192 * (half + 1)]
        y = np.asarray(res.results[cx]["out"]).reshape(8192, 256)
        outf[bi][perm] = y
    return outf



def bench_exec(in_maps, nc, iters=8):
    """Time repeated NEFF executions with device-resident inputs.

    Returns (min_s, all_s). Mirrors bass2jax.run_bass_via_pjrt's multi-core
    path but keeps the jitted callable and input device arrays across calls.
    """
    import time as _time
    import jax
    from jax.sharding import Mesh, PartitionSpec, NamedSharding
    from jax.experimental.shard_map import shard_map
    from concourse import bass2jax, mybir as mb

    bass2jax.install_neuronx_cc_hook()
    n_cores = len(in_maps)
    partition_name = nc.partition_id_tensor.name if nc.partition_id_tensor else None
    in_names, out_names, out_avals, zero_outs = [], [], [], []
    for alloc in nc.m.functions[0].allocations:
        if not isinstance(alloc, mb.MemoryLocationSet):
            continue
        name = alloc.memorylocations[0].name
        if alloc.kind == "ExternalInput":
            if name != partition_name:
                in_names.append(name)
        elif alloc.kind == "ExternalOutput":
            shape = tuple(alloc.tensor_shape)
            dtype = mb.dt.np(alloc.dtype)
            out_names.append(name)
            out_avals.append(jax.core.ShapedArray(shape, dtype))
            zero_outs.append(np.zeros(shape, dtype))
    n_params = len(in_names)
    n_outs = len(out_avals)
    all_in_names = list(in_names) + list(out_names)
    if partition_name is not None:
        all_in_names.append(partition_name)

    def _body(*args):
        operands = list(args)
        if partition_name is not None:
            operands.append(bass2jax.partition_id_tensor())
        outs = bass2jax._bass_exec_p.bind(
            *operands,
            out_avals=tuple(out_avals),
            in_names=tuple(all_in_names),
            out_names=tuple(out_names),
            lowering_input_output_aliases=(),
            sim_require_finite=True,
            sim_require_nnan=True,
            nc=nc,
        )
        return tuple(outs)

    devices = jax.devices()[:n_cores]
    mesh = Mesh(np.asarray(devices), ("core",))
    pspec = PartitionSpec("core")
    sharded = jax.jit(
        shard_map(_body, mesh=mesh, in_specs=(pspec,) * (n_params + n_outs),
                  out_specs=(pspec,) * n_outs, check_rep=False),
        donate_argnums=tuple(range(n_params, n_params + n_outs)),
        keep_unused=True,
    )
    shard = NamedSharding(mesh, pspec)
    concat_in = [
        jax.device_put(
            np.concatenate([np.asarray(in_maps[c][nm]) for c in range(n_cores)], axis=0),
            shard)
        for nm in in_names
    ]
    jax.block_until_ready(concat_in)

    def zeros():
        zs = [jax.device_put(np.zeros((n_cores * z.shape[0], *z.shape[1:]), z.dtype),
                             shard) for z in zero_outs]
        jax.block_until_ready(zs)
        return zs

    times = []
    out = sharded(*concat_in, *zeros())  # warm (compile)
    jax.block_until_ready(out)
    for _ in range(iters):
        zs = zeros()
        t0 = _time.perf_counter()
        out = sharded(*concat_in, *zs)
        jax.block_until_ready(out)
        times.append(_time.perf_counter() - t0)
    return min(times), times


if __name__ == "__main__":
    rng = np.random.default_rng(0)
    qkv = rng.standard_normal((4, 16384, 768), dtype=np.float32)
    sim = rng.standard_normal((4, 16384, 64), dtype=np.float32)
    pw = (rng.standard_normal((256, 256), dtype=np.float32) * 0.02)
    pb = np.zeros(256, dtype=np.float32)
    lsc = np.log(10.0 * np.ones((1, 1), dtype=np.float32))
    o = kernel(qkv=qkv, sim=sim, proj_w=pw, proj_b=pb, logit_scale=lsc)
    print("ran", o.shape, o.dtype)
